# revision 18
# baseline (speedup 1.0000x reference)
"""SRP layer distributed Bass kernel for TRN2 (v7).

Math (full problem): out = Psi_c @ x.T @ x with Psi_c = Psi - rowmean(Psi).
  x [D, N] f32, Psi [O, N] f32, out [O, N] f32  (D=4096, N=8192, O=2048)

Distribution over 8 cores as a 2x4 grid: core c -> (i = c % 2: n-half,
j = c // 2: o-quarter). All heavy layout work happens on the HOST:
Psi is centered (global row mean) and transposed, x is transposed, and
both x layouts are cast to bf16 and pre-tiled so that every device load
is a contiguous [128, 4096] (1 MB) DMA. The device then does NOTHING but
matmuls: no PE transposes, no centering correction.

Per core:
  xt   [4096, 4096] bf16: xT tiled - block b = p*4+g holds rows of
       x_i^T for k-tiles 8g..8g+7 (n), d-panel p (512 cols)
  xn   [4096, 4096] bf16: x natural tiled - block b = ncn*4+g holds
       kd-tiles 8g..8g+7 (d), n-chunk ncn (512 cols)
  psit [512, 4096] bf16: Psi_c^T tiled - block g holds k-tiles 8g..8g+7
       (n) by all 512 o columns
  out  [512, 4096] f32 natural [o, n-half]

Pipeline: mm1 computes tmpT[d, o] = sum_n x[d,n] psi_c[o,n] panel by
panel (stationary = xT blocks, moving = psiT); each finished 512-row
panel is pair-AllReduced immediately (8 x 512 KB chunks, overlapped with
later panels). mm2 (stationary = tmpT blocks from the AR, moving = x
natural) streams n-chunks and writes out f32 directly.
"""

from contextlib import ExitStack

import concourse.bacc as bacc
import concourse.mybir as mybir
import concourse.tile as tile

F32 = mybir.dt.float32
BF = mybir.dt.bfloat16

D = 4096          # d_feat
NL = 4096         # local n (N/2)
OL = 512          # local o (O/4)
NP = 8            # d panels (mm1 output chunks / AR chunks)
NCN = 8           # n chunks (mm2 output chunks)
NG = 4            # k-groups of 8 tiles per 4096-wide sbuf tile
GROUPS = [[0, 1], [2, 3], [4, 5], [6, 7]]


def build_srp_kernel(n_cores=8, groups=GROUPS):
    nc = bacc.Bacc("TRN2", target_bir_lowering=False, debug=False,
                   num_devices=n_cores)
    xt_ext = nc.dram_tensor("xt", [D, 4096], BF, kind="ExternalInput")
    xn_ext = nc.dram_tensor("xn", [D, 4096], BF, kind="ExternalInput")
    psit_ext = nc.dram_tensor("psit", [OL, 4096], BF, kind="ExternalInput")
    out_ext = nc.dram_tensor("out", [OL, NL], F32, kind="ExternalOutput")

    with ExitStack() as stack:
        tc = stack.enter_context(tile.TileContext(nc))
        dram = stack.enter_context(tc.tile_pool(name="dram", bufs=1, space="DRAM"))
        ps = stack.enter_context(tc.tile_pool(name="ps", bufs=1, space="PSUM"))
        sb = stack.enter_context(tc.tile_pool(name="sb", bufs=1))

        tmp_in = [dram.tile([OL, OL], BF, tag=f"tmp_in{p}", bufs=1,
                            name=f"tmp_in{p}")
                  for p in range(NP)]
        tmp_out = [dram.tile([OL, OL], BF, tag=f"tmp_out{p}", bufs=1,
                             name=f"tmp_out{p}")
                   for p in range(NP)]

        # ---- tile dicts + load helpers ----
        xn_tiles = {}

        def xn_load(ncn, eng):
            for g in range(NG):
                t = sb.tile([128, 4096], BF, tag="xn", bufs=6,
                            name=f"xn{ncn}_{g}")
                eng.dma_start(t[:], xn_ext[(ncn * NG + g) * 128:
                                           (ncn * NG + g + 1) * 128, :])
                xn_tiles[(ncn, g)] = t

        xt_tiles = {}

        def xt_tile(p, g):
            t = sb.tile([128, 4096], BF, tag="xt", bufs=8, name=f"xt{p}_{g}")
            xt_tiles[(p, g)] = t
            return t

        def load_cols(eng, dst, src_rows, widths):
            """Load [128, 4096] in column chunks (subtile deps let matmuls
            start on the first chunk)."""
            c0 = 0
            for w in widths:
                eng.dma_start(dst[:, c0:c0 + w], src_rows[:, c0:c0 + w])
                c0 += w

        # ---- startup: panel-0 xt + psiT emitted in consumption order,
        # alternating the two HWDGE rings per tile ----
        psi_sb = [sb.tile([128, 4096], BF, tag="psi", bufs=NG, name=f"psi{g}")
                  for g in range(NG)]
        for g in range(NG):
            xt_eng, psi_eng = (nc.sync, nc.scalar) if g % 2 == 0 else \
                              (nc.scalar, nc.sync)
            widths = [512] * 8 if g == 0 else [1024] * 4
            load_cols(xt_eng, xt_tile(0, g),
                      xt_ext[(0 * NG + g) * 128:(0 * NG + g + 1) * 128, :],
                      widths)
            load_cols(psi_eng, psi_sb[g],
                      psit_ext[g * 128:(g + 1) * 128, :], widths)

        def xt_load(p):
            for g in range(NG):
                eng = nc.sync if g % 2 == 0 else nc.scalar
                row = (p * NG + g) * 128
                eng.dma_start(xt_tile(p, g)[:], xt_ext[row:row + 128, :])

        xt_load(1)
        for p in range(NP):
            if p + 2 < NP:
                xt_load(p + 2)
            # g-major order: each (xt, psi) tile pair is consumed in one
            # contiguous 32-matmul burst (2 MB / 6.8 us = streamable at
            # ~300 GB/s), with the panel's 4 accumulation groups open
            # simultaneously in 4 PSUM banks.
            pts = [ps.tile([128, 512], F32, tag="ps", bufs=8,
                           name=f"mm1_{p}_{dtl}")
                   for dtl in range(4)]
            for g in range(NG):
                xt = xt_tiles[(p, g)]
                for kk in range(8):
                    k = 8 * g + kk
                    for dtl in range(4):
                        nc.tensor.matmul(
                            pts[dtl][:],
                            xt[:, kk * 512 + dtl * 128:
                               kk * 512 + (dtl + 1) * 128],
                            psi_sb[g][:, kk * 512:(kk + 1) * 512],
                            start=(k == 0), stop=(k == 31))
            for dtl in range(4):
                st = sb.tile([128, 512], BF, tag="st", bufs=4,
                             name=f"st{p}_{dtl}")
                nc.vector.tensor_copy(st[:], pts[dtl][:])
                nc.scalar.dma_start(
                    tmp_in[p][dtl * 128:(dtl + 1) * 128, :], st[:])
            nc.gpsimd.collective_compute(
                "AllReduce", mybir.AluOpType.add, replica_groups=groups,
                ins=[tmp_in[p].opt()], outs=[tmp_out[p].opt()])
            if p == 1:
                # xn chunk 0 (needed at the mm1->mm2 edge): desc-gen on the
                # otherwise-idle gpsimd ring, clear of the startup HBM burst
                xn_load(0, nc.gpsimd)

        # ---- mm2 stationary tiles (tmpT summed) on sync after xt ----
        ts_tiles = []
        for p in range(NP):
            for j in range(4):
                t = sb.tile([128, 512], BF, tag="ts", bufs=32,
                            name=f"ts{p}_{j}")
                nc.sync.dma_start(t[:], tmp_out[p][j * 128:(j + 1) * 128, :])
                ts_tiles.append(t)

        # ---- remaining xn loads on gpsimd (after all AR triggers) ----
        for ncn in range(1, NCN):
            xn_load(ncn, nc.gpsimd)

        # ---- mm2: out[o, n] = tmpT^T @ x ----
        for ncn in range(NCN):
            mm = [ps.tile([128, 512], F32, tag="ps", bufs=8,
                          name=f"mm2_{ncn}_{ot}")
                  for ot in range(4)]
            for g in range(NG):
                xnt = xn_tiles[(ncn, g)]
                for kk in range(8):
                    kd = 8 * g + kk
                    for ot in range(4):
                        nc.tensor.matmul(
                            mm[ot][:],
                            ts_tiles[kd][:, ot * 128:(ot + 1) * 128],
                            xnt[:, kk * 512:(kk + 1) * 512],
                            start=(kd == 0), stop=(kd == 31))
            for ot in range(4):
                os_ = sb.tile([128, 512], F32, tag="os", bufs=8,
                              name=f"os{ncn}_{ot}")
                nc.vector.tensor_copy(os_[:], mm[ot][:])
                eng = nc.scalar if ot % 2 == 0 else nc.sync
                eng.dma_start(
                    out_ext[ot * 128:(ot + 1) * 128,
                            ncn * 512:(ncn + 1) * 512], os_[:])
    nc.compile()
    return nc


# ---------------- host-side sharding / tiling ----------------
import numpy as np
import ml_dtypes

BF_NP = ml_dtypes.bfloat16

D_FULL, N_FULL, O_FULL = 4096, 8192, 2048
N_CORES = 8


def _tile_k_major(a_bf):
    """[4096 rows, C cols] -> blocks of [128, 8*C'] with k-tiles grouped 8
    per block: in[(8g+kk)*128 + r, c] -> out[(b, r, kk*C512 + c)] per 512-col
    chunk. Works for both xT (chunk axis = d panels) and x natural (chunk
    axis = n chunks).
    Input must be [4096, 4096]. Output [32*128, 4096]."""
    A5 = a_bf.reshape(4, 8, 128, 8, 512)         # g, kk, r, chunk, c
    B = A5.transpose(3, 0, 2, 1, 4)              # chunk, g, r, kk, c
    return np.ascontiguousarray(B).reshape(4096, 4096)


def _tile_psit(psit_bf):
    """[4096, 512] -> [512, 4096]: block g = [128 r, 8 kk * 512 oc]."""
    P4 = psit_bf.reshape(4, 8, 128, 512)         # g, kk, r, oc
    Q = P4.transpose(0, 2, 1, 3)                 # g, r, kk, oc
    return np.ascontiguousarray(Q).reshape(512, 4096)


def make_in_maps(x, Psi, n_cores=8):
    psi_c = (Psi.astype(np.float64)
             - Psi.astype(np.float64).mean(axis=1, keepdims=True))
    psi_c = psi_c.astype(np.float32)
    in_maps = []
    for c in range(n_cores):
        i, j = c % 2, c // 2
        xs = x[:, i * NL:(i + 1) * NL].astype(BF_NP)          # [D, NL]
        xT = np.ascontiguousarray(xs.T)                        # [NL, D]
        ps_ = psi_c[j * OL:(j + 1) * OL, i * NL:(i + 1) * NL].astype(BF_NP)
        psT = np.ascontiguousarray(ps_.T)                      # [NL, OL]
        in_maps.append({
            "xt": _tile_k_major(xT),
            "xn": _tile_k_major(xs),
            "psit": _tile_psit(psT),
        })
    return in_maps


# ---------------- harness-facing wrapper ----------------
_NC_CACHE = {}


def _get_nc():
    if "nc" not in _NC_CACHE:
        _NC_CACHE["nc"] = build_srp_kernel(n_cores=N_CORES, groups=GROUPS)
    return _NC_CACHE["nc"]


def kernel(x, Psi):
    """out = (Psi - rowmean(Psi)) @ x.T @ x on 8 TRN2 NeuronCores."""
    from concourse.bass_utils import run_bass_kernel_spmd
    x = np.asarray(x, dtype=np.float32)
    Psi = np.asarray(Psi, dtype=np.float32)
    assert x.shape == (D_FULL, N_FULL) and Psi.shape == (O_FULL, N_FULL)
    nc = _get_nc()
    in_maps = make_in_maps(x, Psi, n_cores=N_CORES)
    res = run_bass_kernel_spmd(nc, in_maps, core_ids=list(range(N_CORES)))
    out = np.empty((O_FULL, N_FULL), dtype=np.float32)
    for c in range(N_CORES):
        i, j = c % 2, c // 2
        out[j * OL:(j + 1) * OL, i * NL:(i + 1) * NL] = res.results[c]["out"]
    return out


# revision 23
# speedup vs baseline: 1.0280x; 1.0280x over previous
"""SRP layer distributed Bass kernel for TRN2 (v7).

Math (full problem): out = Psi_c @ x.T @ x with Psi_c = Psi - rowmean(Psi).
  x [D, N] f32, Psi [O, N] f32, out [O, N] f32  (D=4096, N=8192, O=2048)

Distribution over 8 cores as a 2x4 grid: core c -> (i = c % 2: n-half,
j = c // 2: o-quarter). All heavy layout work happens on the HOST:
Psi is centered (global row mean) and transposed, x is transposed, and
both x layouts are cast to bf16 and pre-tiled so that every device load
is a contiguous [128, 4096] (1 MB) DMA. The device then does NOTHING but
matmuls: no PE transposes, no centering correction.

Per core:
  xt   [4096, 4096] bf16: xT tiled - block b = p*4+g holds rows of
       x_i^T for k-tiles 8g..8g+7 (n), d-panel p (512 cols)
  xn   [4096, 4096] bf16: x natural tiled - block b = ncn*4+g holds
       kd-tiles 8g..8g+7 (d), n-chunk ncn (512 cols)
  psit [512, 4096] bf16: Psi_c^T tiled - block g holds k-tiles 8g..8g+7
       (n) by all 512 o columns
  out  [512, 4096] f32 natural [o, n-half]

Pipeline: mm1 computes tmpT[d, o] = sum_n x[d,n] psi_c[o,n] panel by
panel (stationary = xT blocks, moving = psiT); each finished 512-row
panel is pair-AllReduced immediately (8 x 512 KB chunks, overlapped with
later panels). mm2 (stationary = tmpT blocks from the AR, moving = x
natural) streams n-chunks and writes out f32 directly.
"""

from contextlib import ExitStack

import concourse.bacc as bacc
import concourse.mybir as mybir
import concourse.tile as tile

F32 = mybir.dt.float32
BF = mybir.dt.bfloat16

D = 4096          # d_feat
NL = 4096         # local n (N/2)
OL = 512          # local o (O/4)
NP = 8            # d panels (mm1 output chunks / AR chunks)
NCN = 8           # n chunks (mm2 output chunks)
NG = 4            # k-groups of 8 tiles per 4096-wide sbuf tile
GROUPS = [[0, 1], [2, 3], [4, 5], [6, 7]]


def build_srp_kernel(n_cores=8, groups=GROUPS):
    nc = bacc.Bacc("TRN2", target_bir_lowering=False, debug=False,
                   num_devices=n_cores)
    xt_ext = nc.dram_tensor("xt", [D, 4096], BF, kind="ExternalInput")
    xn_ext = nc.dram_tensor("xn", [D, 4096], BF, kind="ExternalInput")
    psit_ext = nc.dram_tensor("psit", [OL, 4096], BF, kind="ExternalInput")
    out_ext = nc.dram_tensor("out", [OL, NL], F32, kind="ExternalOutput")

    with ExitStack() as stack:
        tc = stack.enter_context(tile.TileContext(nc))
        dram = stack.enter_context(tc.tile_pool(name="dram", bufs=1, space="DRAM"))
        ps = stack.enter_context(tc.tile_pool(name="ps", bufs=1, space="PSUM"))
        sb = stack.enter_context(tc.tile_pool(name="sb", bufs=1))

        tmp_in = [dram.tile([OL, OL], BF, tag=f"tmp_in{p}", bufs=1,
                            name=f"tmp_in{p}")
                  for p in range(NP)]
        tmp_out = [dram.tile([OL, OL], BF, tag=f"tmp_out{p}", bufs=1,
                             name=f"tmp_out{p}")
                   for p in range(NP)]

        # ---- tile dicts + load helpers ----
        xn_tiles = {}

        def xn_load(ncn, eng):
            for g in range(NG):
                t = sb.tile([128, 4096], BF, tag="xn", bufs=8,
                            name=f"xn{ncn}_{g}")
                eng.dma_start(t[:], xn_ext[(ncn * NG + g) * 128:
                                           (ncn * NG + g + 1) * 128, :])
                xn_tiles[(ncn, g)] = t

        xt_tiles = {}

        def xt_tile(p, g):
            t = sb.tile([128, 4096], BF, tag="xt", bufs=6, name=f"xt{p}_{g}")
            xt_tiles[(p, g)] = t
            return t

        def load_cols(eng, dst, src_rows, widths):
            """Load [128, 4096] in column chunks (subtile deps let matmuls
            start on the first chunk)."""
            c0 = 0
            for w in widths:
                eng.dma_start(dst[:, c0:c0 + w], src_rows[:, c0:c0 + w])
                c0 += w

        # ---- startup: panel-0 xt + psiT emitted in consumption order,
        # alternating the two HWDGE rings per tile ----
        psi_sb = [sb.tile([128, 4096], BF, tag="psi", bufs=NG, name=f"psi{g}")
                  for g in range(NG)]
        for g in range(NG):
            xt_eng, psi_eng = (nc.sync, nc.scalar) if g % 2 == 0 else \
                              (nc.scalar, nc.sync)
            widths = [1024] * 4 if g == 0 else [2048] * 2
            load_cols(xt_eng, xt_tile(0, g),
                      xt_ext[(0 * NG + g) * 128:(0 * NG + g + 1) * 128, :],
                      widths)
            load_cols(psi_eng, psi_sb[g],
                      psit_ext[g * 128:(g + 1) * 128, :], widths)

        def xt_load(p):
            for g in range(NG):
                eng = nc.sync if g % 2 == 0 else nc.scalar
                row = (p * NG + g) * 128
                eng.dma_start(xt_tile(p, g)[:], xt_ext[row:row + 128, :])

        xt_load(1)
        for p in range(NP):
            if p + 2 < NP:
                xt_load(p + 2)
            # g-major order: each (xt, psi) tile pair is consumed in one
            # contiguous 32-matmul burst (2 MB / 6.8 us = streamable at
            # ~300 GB/s), with the panel's 4 accumulation groups open
            # simultaneously in 4 PSUM banks.
            pts = [ps.tile([128, 512], F32, tag="ps", bufs=8,
                           name=f"mm1_{p}_{dtl}")
                   for dtl in range(4)]
            for g in range(NG):
                xt = xt_tiles[(p, g)]
                for kk in range(8):
                    k = 8 * g + kk
                    for dtl in range(4):
                        nc.tensor.matmul(
                            pts[dtl][:],
                            xt[:, kk * 512 + dtl * 128:
                               kk * 512 + (dtl + 1) * 128],
                            psi_sb[g][:, kk * 512:(kk + 1) * 512],
                            start=(k == 0), stop=(k == 31))
            for dtl in range(4):
                st = sb.tile([128, 512], BF, tag="st", bufs=4,
                             name=f"st{p}_{dtl}")
                nc.vector.tensor_copy(st[:], pts[dtl][:])
                nc.scalar.dma_start(
                    tmp_in[p][dtl * 128:(dtl + 1) * 128, :], st[:])
            nc.gpsimd.collective_compute(
                "AllReduce", mybir.AluOpType.add, replica_groups=groups,
                ins=[tmp_in[p].opt()], outs=[tmp_out[p].opt()])
            if p == 1:
                # xn chunks 0/1 (needed right at the mm1->mm2 edge) load on
                # scalar during mm1, clear of the startup HBM burst; they
                # fill the xn pool (bufs=8) exactly, so no alloc stalls.
                xn_load(0, nc.scalar)
            elif p == 3:
                xn_load(1, nc.scalar)

        # ---- mm2 stationary tiles (tmpT summed) on sync after xt ----
        ts_tiles = []
        for p in range(NP):
            for j in range(4):
                t = sb.tile([128, 512], BF, tag="ts", bufs=32,
                            name=f"ts{p}_{j}")
                nc.sync.dma_start(t[:], tmp_out[p][j * 128:(j + 1) * 128, :])
                ts_tiles.append(t)

        # ---- remaining xn loads on gpsimd (after all AR triggers) ----
        for ncn in range(2, NCN):
            xn_load(ncn, nc.gpsimd)

        # ---- mm2: out[o, n] = tmpT^T @ x ----
        for ncn in range(NCN):
            mm = [ps.tile([128, 512], F32, tag="ps", bufs=8,
                          name=f"mm2_{ncn}_{ot}")
                  for ot in range(4)]
            for g in range(NG):
                xnt = xn_tiles[(ncn, g)]
                for kk in range(8):
                    kd = 8 * g + kk
                    for ot in range(4):
                        nc.tensor.matmul(
                            mm[ot][:],
                            ts_tiles[kd][:, ot * 128:(ot + 1) * 128],
                            xnt[:, kk * 512:(kk + 1) * 512],
                            start=(kd == 0), stop=(kd == 31))
            for ot in range(4):
                os_ = sb.tile([128, 512], F32, tag="os", bufs=8,
                              name=f"os{ncn}_{ot}")
                nc.vector.tensor_copy(os_[:], mm[ot][:])
                eng = nc.scalar if ot % 2 == 0 else nc.sync
                eng.dma_start(
                    out_ext[ot * 128:(ot + 1) * 128,
                            ncn * 512:(ncn + 1) * 512], os_[:])
    nc.compile()
    return nc


# ---------------- host-side sharding / tiling ----------------
import numpy as np
import ml_dtypes

BF_NP = ml_dtypes.bfloat16

D_FULL, N_FULL, O_FULL = 4096, 8192, 2048
N_CORES = 8


def _tile_k_major(a_bf):
    """[4096 rows, C cols] -> blocks of [128, 8*C'] with k-tiles grouped 8
    per block: in[(8g+kk)*128 + r, c] -> out[(b, r, kk*C512 + c)] per 512-col
    chunk. Works for both xT (chunk axis = d panels) and x natural (chunk
    axis = n chunks).
    Input must be [4096, 4096]. Output [32*128, 4096]."""
    A5 = a_bf.reshape(4, 8, 128, 8, 512)         # g, kk, r, chunk, c
    B = A5.transpose(3, 0, 2, 1, 4)              # chunk, g, r, kk, c
    return np.ascontiguousarray(B).reshape(4096, 4096)


def _tile_psit(psit_bf):
    """[4096, 512] -> [512, 4096]: block g = [128 r, 8 kk * 512 oc]."""
    P4 = psit_bf.reshape(4, 8, 128, 512)         # g, kk, r, oc
    Q = P4.transpose(0, 2, 1, 3)                 # g, r, kk, oc
    return np.ascontiguousarray(Q).reshape(512, 4096)


def make_in_maps(x, Psi, n_cores=8):
    psi_c = (Psi.astype(np.float64)
             - Psi.astype(np.float64).mean(axis=1, keepdims=True))
    psi_c = psi_c.astype(np.float32)
    in_maps = []
    for c in range(n_cores):
        i, j = c % 2, c // 2
        xs = x[:, i * NL:(i + 1) * NL].astype(BF_NP)          # [D, NL]
        xT = np.ascontiguousarray(xs.T)                        # [NL, D]
        ps_ = psi_c[j * OL:(j + 1) * OL, i * NL:(i + 1) * NL].astype(BF_NP)
        psT = np.ascontiguousarray(ps_.T)                      # [NL, OL]
        in_maps.append({
            "xt": _tile_k_major(xT),
            "xn": _tile_k_major(xs),
            "psit": _tile_psit(psT),
        })
    return in_maps


# ---------------- harness-facing wrapper ----------------
_NC_CACHE = {}


def _get_nc():
    if "nc" not in _NC_CACHE:
        _NC_CACHE["nc"] = build_srp_kernel(n_cores=N_CORES, groups=GROUPS)
    return _NC_CACHE["nc"]


def kernel(x, Psi):
    """out = (Psi - rowmean(Psi)) @ x.T @ x on 8 TRN2 NeuronCores."""
    from concourse.bass_utils import run_bass_kernel_spmd
    x = np.asarray(x, dtype=np.float32)
    Psi = np.asarray(Psi, dtype=np.float32)
    assert x.shape == (D_FULL, N_FULL) and Psi.shape == (O_FULL, N_FULL)
    nc = _get_nc()
    in_maps = make_in_maps(x, Psi, n_cores=N_CORES)
    res = run_bass_kernel_spmd(nc, in_maps, core_ids=list(range(N_CORES)))
    out = np.empty((O_FULL, N_FULL), dtype=np.float32)
    for c in range(N_CORES):
        i, j = c % 2, c // 2
        out[j * OL:(j + 1) * OL, i * NL:(i + 1) * NL] = res.results[c]["out"]
    return out


# revision 26
# speedup vs baseline: 1.0569x; 1.0281x over previous
"""SRP layer distributed Bass kernel for TRN2 (v7).

Math (full problem): out = Psi_c @ x.T @ x with Psi_c = Psi - rowmean(Psi).
  x [D, N] f32, Psi [O, N] f32, out [O, N] f32  (D=4096, N=8192, O=2048)

Distribution over 8 cores as a 2x4 grid: core c -> (i = c % 2: n-half,
j = c // 2: o-quarter). All heavy layout work happens on the HOST:
Psi is centered (global row mean) and transposed, x is transposed, and
both x layouts are cast to bf16 and pre-tiled so that every device load
is a contiguous [128, 4096] (1 MB) DMA. The device then does NOTHING but
matmuls: no PE transposes, no centering correction.

Per core:
  xt   [4096, 4096] bf16: xT tiled - block b = p*4+g holds rows of
       x_i^T for k-tiles 8g..8g+7 (n), d-panel p (512 cols)
  xn   [4096, 4096] bf16: x natural tiled - block b = ncn*4+g holds
       kd-tiles 8g..8g+7 (d), n-chunk ncn (512 cols)
  psit [512, 4096] bf16: Psi_c^T tiled - block g holds k-tiles 8g..8g+7
       (n) by all 512 o columns
  out  [512, 4096] f32 natural [o, n-half]

Pipeline: mm1 computes tmpT[d, o] = sum_n x[d,n] psi_c[o,n] panel by
panel (stationary = xT blocks, moving = psiT); each finished 512-row
panel is pair-AllReduced immediately (8 x 512 KB chunks, overlapped with
later panels). mm2 (stationary = tmpT blocks from the AR, moving = x
natural) streams n-chunks and writes out f32 directly.
"""

from contextlib import ExitStack

import concourse.bacc as bacc
import concourse.mybir as mybir
import concourse.tile as tile

F32 = mybir.dt.float32
BF = mybir.dt.bfloat16

D = 4096          # d_feat
NL = 4096         # local n (N/2)
OL = 512          # local o (O/4)
NP = 8            # d panels (mm1 output chunks / AR chunks)
NCN = 8           # n chunks (mm2 output chunks)
NG = 4            # k-groups of 8 tiles per 4096-wide sbuf tile
GROUPS = [[0, 1], [2, 3], [4, 5], [6, 7]]


def build_srp_kernel(n_cores=8, groups=GROUPS):
    nc = bacc.Bacc("TRN2", target_bir_lowering=False, debug=False,
                   num_devices=n_cores)
    xt_ext = nc.dram_tensor("xt", [D, 4096], BF, kind="ExternalInput")
    xn_ext = nc.dram_tensor("xn", [D, 4096], BF, kind="ExternalInput")
    psit_ext = nc.dram_tensor("psit", [OL, 4096], BF, kind="ExternalInput")
    out_ext = nc.dram_tensor("out", [OL, NL], F32, kind="ExternalOutput")

    with ExitStack() as stack:
        tc = stack.enter_context(tile.TileContext(nc))
        dram = stack.enter_context(tc.tile_pool(name="dram", bufs=1, space="DRAM"))
        ps = stack.enter_context(tc.tile_pool(name="ps", bufs=1, space="PSUM"))
        sb = stack.enter_context(tc.tile_pool(name="sb", bufs=1))

        tmp_in = [dram.tile([OL, OL], BF, tag=f"tmp_in{p}", bufs=1,
                            name=f"tmp_in{p}")
                  for p in range(NP)]
        tmp_out = [dram.tile([OL, OL], BF, tag=f"tmp_out{p}", bufs=1,
                             name=f"tmp_out{p}")
                   for p in range(NP)]

        # ---- tile dicts + load helpers ----
        xn_tiles = {}

        def xn_load_one(ncn, g, eng):
            t = sb.tile([128, 4096], BF, tag="xn", bufs=8,
                        name=f"xn{ncn}_{g}")
            eng.dma_start(t[:], xn_ext[(ncn * NG + g) * 128:
                                       (ncn * NG + g + 1) * 128, :])
            xn_tiles[(ncn, g)] = t

        def xn_load(ncn, eng):
            for g in range(NG):
                xn_load_one(ncn, g, eng)

        xt_tiles = {}

        def xt_tile(p, g):
            t = sb.tile([128, 4096], BF, tag="xt", bufs=6, name=f"xt{p}_{g}")
            xt_tiles[(p, g)] = t
            return t

        def load_cols(eng, dst, src_rows, widths):
            """Load [128, 4096] in column chunks (subtile deps let matmuls
            start on the first chunk)."""
            c0 = 0
            for w in widths:
                eng.dma_start(dst[:, c0:c0 + w], src_rows[:, c0:c0 + w])
                c0 += w

        # ---- startup: panel-0 xt + psiT emitted in consumption order,
        # alternating the two HWDGE rings per tile ----
        psi_sb = [sb.tile([128, 4096], BF, tag="psi", bufs=NG, name=f"psi{g}")
                  for g in range(NG)]
        for g in range(NG):
            xt_eng, psi_eng = (nc.sync, nc.scalar) if g % 2 == 0 else \
                              (nc.scalar, nc.sync)
            widths = [512, 512, 1024, 2048] if g == 0 else [2048] * 2
            load_cols(xt_eng, xt_tile(0, g),
                      xt_ext[(0 * NG + g) * 128:(0 * NG + g + 1) * 128, :],
                      widths)
            load_cols(psi_eng, psi_sb[g],
                      psit_ext[g * 128:(g + 1) * 128, :], widths)

        def xt_load(p):
            for g in range(NG):
                eng = nc.sync if g % 2 == 0 else nc.scalar
                row = (p * NG + g) * 128
                eng.dma_start(xt_tile(p, g)[:], xt_ext[row:row + 128, :])

        xt_load(1)
        for p in range(NP):
            if p + 2 < NP:
                xt_load(p + 2)
            # g-major order: each (xt, psi) tile pair is consumed in one
            # contiguous 32-matmul burst (2 MB / 6.8 us = streamable at
            # ~300 GB/s), with the panel's 4 accumulation groups open
            # simultaneously in 4 PSUM banks.
            pts = [ps.tile([128, 512], F32, tag="ps", bufs=8,
                           name=f"mm1_{p}_{dtl}")
                   for dtl in range(4)]
            for g in range(NG):
                xt = xt_tiles[(p, g)]
                for kk in range(8):
                    k = 8 * g + kk
                    for dtl in range(4):
                        nc.tensor.matmul(
                            pts[dtl][:],
                            xt[:, kk * 512 + dtl * 128:
                               kk * 512 + (dtl + 1) * 128],
                            psi_sb[g][:, kk * 512:(kk + 1) * 512],
                            start=(k == 0), stop=(k == 31))
            for dtl in range(4):
                st = sb.tile([128, 512], BF, tag="st", bufs=4,
                             name=f"st{p}_{dtl}")
                nc.vector.tensor_copy(st[:], pts[dtl][:])
                nc.scalar.dma_start(
                    tmp_in[p][dtl * 128:(dtl + 1) * 128, :], st[:])
            nc.gpsimd.collective_compute(
                "AllReduce", mybir.AluOpType.add, replica_groups=groups,
                ins=[tmp_in[p].opt()], outs=[tmp_out[p].opt()])
            # xn chunks 0/1 (needed right at the mm1->mm2 edge) trickle in
            # on scalar during mm1 - one 1 MB tile per panel so they never
            # displace a panel's xt tiles; 8 tiles fill the xn pool exactly.
            xn_load_one(p // 4, p % 4, nc.scalar)

        # ---- mm2 stationary tiles (tmpT summed) on sync after xt ----
        ts_tiles = []
        for p in range(NP):
            for j in range(4):
                t = sb.tile([128, 512], BF, tag="ts", bufs=32,
                            name=f"ts{p}_{j}")
                nc.sync.dma_start(t[:], tmp_out[p][j * 128:(j + 1) * 128, :])
                ts_tiles.append(t)

        # ---- remaining xn loads on gpsimd (after all AR triggers) ----
        for ncn in range(2, NCN):
            xn_load(ncn, nc.gpsimd)

        # ---- mm2: out[o, n] = tmpT^T @ x ----
        for ncn in range(NCN):
            mm = [ps.tile([128, 512], F32, tag="ps", bufs=8,
                          name=f"mm2_{ncn}_{ot}")
                  for ot in range(4)]
            for g in range(NG):
                xnt = xn_tiles[(ncn, g)]
                for kk in range(8):
                    kd = 8 * g + kk
                    for ot in range(4):
                        nc.tensor.matmul(
                            mm[ot][:],
                            ts_tiles[kd][:, ot * 128:(ot + 1) * 128],
                            xnt[:, kk * 512:(kk + 1) * 512],
                            start=(kd == 0), stop=(kd == 31))
            for ot in range(4):
                os_ = sb.tile([128, 512], F32, tag="os", bufs=8,
                              name=f"os{ncn}_{ot}")
                nc.vector.tensor_copy(os_[:], mm[ot][:])
                eng = nc.scalar if ot % 2 == 0 else nc.sync
                eng.dma_start(
                    out_ext[ot * 128:(ot + 1) * 128,
                            ncn * 512:(ncn + 1) * 512], os_[:])
    nc.compile()
    return nc


# ---------------- host-side sharding / tiling ----------------
import numpy as np
import ml_dtypes

BF_NP = ml_dtypes.bfloat16

D_FULL, N_FULL, O_FULL = 4096, 8192, 2048
N_CORES = 8


def _tile_k_major(a_bf):
    """[4096 rows, C cols] -> blocks of [128, 8*C'] with k-tiles grouped 8
    per block: in[(8g+kk)*128 + r, c] -> out[(b, r, kk*C512 + c)] per 512-col
    chunk. Works for both xT (chunk axis = d panels) and x natural (chunk
    axis = n chunks).
    Input must be [4096, 4096]. Output [32*128, 4096]."""
    A5 = a_bf.reshape(4, 8, 128, 8, 512)         # g, kk, r, chunk, c
    B = A5.transpose(3, 0, 2, 1, 4)              # chunk, g, r, kk, c
    return np.ascontiguousarray(B).reshape(4096, 4096)


def _tile_psit(psit_bf):
    """[4096, 512] -> [512, 4096]: block g = [128 r, 8 kk * 512 oc]."""
    P4 = psit_bf.reshape(4, 8, 128, 512)         # g, kk, r, oc
    Q = P4.transpose(0, 2, 1, 3)                 # g, r, kk, oc
    return np.ascontiguousarray(Q).reshape(512, 4096)


def make_in_maps(x, Psi, n_cores=8):
    psi_c = (Psi.astype(np.float64)
             - Psi.astype(np.float64).mean(axis=1, keepdims=True))
    psi_c = psi_c.astype(np.float32)
    in_maps = []
    for c in range(n_cores):
        i, j = c % 2, c // 2
        xs = x[:, i * NL:(i + 1) * NL].astype(BF_NP)          # [D, NL]
        xT = np.ascontiguousarray(xs.T)                        # [NL, D]
        ps_ = psi_c[j * OL:(j + 1) * OL, i * NL:(i + 1) * NL].astype(BF_NP)
        psT = np.ascontiguousarray(ps_.T)                      # [NL, OL]
        in_maps.append({
            "xt": _tile_k_major(xT),
            "xn": _tile_k_major(xs),
            "psit": _tile_psit(psT),
        })
    return in_maps


# ---------------- harness-facing wrapper ----------------
_NC_CACHE = {}


def _get_nc():
    if "nc" not in _NC_CACHE:
        _NC_CACHE["nc"] = build_srp_kernel(n_cores=N_CORES, groups=GROUPS)
    return _NC_CACHE["nc"]


def kernel(x, Psi):
    """out = (Psi - rowmean(Psi)) @ x.T @ x on 8 TRN2 NeuronCores."""
    from concourse.bass_utils import run_bass_kernel_spmd
    x = np.asarray(x, dtype=np.float32)
    Psi = np.asarray(Psi, dtype=np.float32)
    assert x.shape == (D_FULL, N_FULL) and Psi.shape == (O_FULL, N_FULL)
    nc = _get_nc()
    in_maps = make_in_maps(x, Psi, n_cores=N_CORES)
    res = run_bass_kernel_spmd(nc, in_maps, core_ids=list(range(N_CORES)))
    out = np.empty((O_FULL, N_FULL), dtype=np.float32)
    for c in range(N_CORES):
        i, j = c % 2, c // 2
        out[j * OL:(j + 1) * OL, i * NL:(i + 1) * NL] = res.results[c]["out"]
    return out
